# revision 1
# baseline (speedup 1.0000x reference)
"""Complex-valued causal attention on 8 trn2 NeuronCores.

nn_ComplexAttention: B=2, L=2048, D=1024, H=16 heads (hd=64), fp32 I/O.

Sharding (per the batch+head hint): core c owns batch b = c//4 and heads
4*(c%4) .. 4*(c%4)+3.  Data parallel over B (2 groups of 4 cores), tensor
parallel over heads within a group.  After per-head attention the 4 cores of
a group AllGather the (d-major, fp16) attention outputs and each computes a
256-column slice of the output projection, so the only collective is a 2 MB
AllGather per core.

All on-chip math uses fp16 operands with fp32 PSUM accumulation (fp16 keeps
8 more mantissa bits than bf16 at the same PE/DVE/DMA cost).  Everything is
formulated transposed (d-major) so no activation transposes are needed:

  Qc^T[h] = Wq_eff^T @ xc^T        (xc^T = [x_real^T ; x_imag^T], host-prepped)
  S^T     = Kc^T-block^T @ Qc^T    (real part of complex dot product, both
                                    r/i folded into the 128-deep contraction)
  w^T     = exp(SCALE * S^T)       (no max-subtraction needed: |scores| <~ 8)
  O^T     = V-block^T @ w^T        (V seq-major via 128x128 PE transposes)
  sums    = ones^T @ w^T           (softmax denominators via matmul)
  y^T     = Wo_eff^T @ oc^T        (oc^T = AllGather of all heads' O^T)

The complex arithmetic (4 real matmuls per complex one) is folded into the
host-assembled W_eff matrices with +-W_r/W_i blocks.
"""

import sys

if "/opt/trn_rl_repo" not in sys.path:
    sys.path.insert(0, "/opt/trn_rl_repo")

import numpy as np
import ml_dtypes

import concourse.mybir as mybir
import concourse.tile as tile
from concourse import bacc
from concourse.bass_utils import run_bass_kernel_spmd

B, L, D, H = 2, 2048, 1024, 16
HD = D // H            # 64
SCALE = HD ** (-0.5)
NCORES = 8
GROUP = 4              # cores per batch group
NH = H // GROUP        # 4 local heads per core
JC = NH * 2 * HD       # 512 local projection cols (r+i interleaved by head)
DD = 2 * D             # 2048 stacked (real; imag) contraction dim
F16 = mybir.dt.float16
F32 = mybir.dt.float32

_CACHE = {}


def _build(seq_len=L, repeat=1, with_cc=True, phases="ABC", compile=True):
    """Build + compile the SPMD kernel (identical program on all 8 cores).

    repeat>1 wraps the whole body in a hardware For_i loop (timing variant,
    collective skipped since collectives cannot sit inside control flow).
    """
    from contextlib import nullcontext
    LL = seq_len
    NLC = LL // 512        # l-chunks of 512
    NKB = LL // 128        # k-blocks of 128
    NDD = DD // 128        # contraction chunks (16)

    nc = bacc.Bacc("TRN2", target_bir_lowering=False, debug=False,
                   num_devices=NCORES)

    xcT = nc.dram_tensor("xcT", [DD, LL], F16, kind="ExternalInput")
    wq = nc.dram_tensor("wq", [DD, JC], F16, kind="ExternalInput")
    wk = nc.dram_tensor("wk", [DD, JC], F16, kind="ExternalInput")
    wv = nc.dram_tensor("wv", [DD, JC], F16, kind="ExternalInput")
    wo = nc.dram_tensor("wo", [DD, JC], F16, kind="ExternalInput")
    bo = nc.dram_tensor("bo", [JC, 1], F32, kind="ExternalInput")
    mask = nc.dram_tensor("mask", [128, 128], F16, kind="ExternalInput")
    ident = nc.dram_tensor("ident", [128, 128], F16, kind="ExternalInput")
    ones = nc.dram_tensor("ones", [128, 1], F16, kind="ExternalInput")
    yT = nc.dram_tensor("yT", [JC, LL], F32, kind="ExternalOutput")

    with tile.TileContext(nc) as tc:
        with (
            tc.tile_pool(name="const", bufs=1) as const,
            tc.tile_pool(name="dram", bufs=1, space="DRAM") as dram,
        ):
            mask_t = const.tile([128, 128], F16, tag="mask", name="mask")
            nc.sync.dma_start(mask_t[:], mask[:])
            ident_t = const.tile([128, 128], F16, tag="ident", name="ident")
            nc.sync.dma_start(ident_t[:], ident[:])
            ones_t = const.tile([128, 1], F16, tag="ones", name="ones")
            nc.sync.dma_start(ones_t[:], ones[:])
            bo_t = const.tile([128, NH], F32, tag="bo", name="bo")
            for m in range(NH):
                nc.sync.dma_start(bo_t[:, m:m + 1], bo[m * 128:(m + 1) * 128, :])

            ag_in = dram.tile([JC, LL], F16, tag="ag_in", name="ag_in")
            ag_out = dram.tile([DD, LL], F16, tag="ag_out", name="ag_out")

            _hint = (mybir.EngineType.PE, mybir.EngineType.Activation,
                     mybir.EngineType.DVE, mybir.EngineType.SP,
                     mybir.EngineType.Pool)
            loop_cm = (tc.For_i(0, repeat, 1, hint_engines=_hint)
                       if repeat > 1 else nullcontext())
            with loop_cm:
                with (
                    tc.tile_pool(name="qkvout", bufs=1) as qkvout,
                    tc.tile_pool(name="vpool", bufs=1) as vpool,
                ):
                    qT = [qkvout.tile([128, LL], F16, tag=f"qT{h}", name=f"qT{h}") for h in range(NH)]
                    kT = [qkvout.tile([128, LL], F16, tag=f"kT{h}", name=f"kT{h}") for h in range(NH)]
                    vv = [vpool.tile([128, LL], F16, tag=f"v{h}", name=f"v{h}") for h in range(NH)]

                    # ---------------- Phase A: projections + V transpose ----------
                    with (
                        tc.tile_pool(name="xw", bufs=1) as xw,
                        tc.tile_pool(name="vt", bufs=1) as vt,
                        tc.tile_pool(name="pps", bufs=3, space="PSUM") as pps,
                        tc.tile_pool(name="vtps", bufs=2, space="PSUM") as vtps,
                    ):
                        xcT_t = []
                        for k in range(NDD):
                            t = xw.tile([128, LL], F16, tag=f"xcT{k}", name=f"xcT{k}")
                            nc.sync.dma_start(t[:], xcT[k * 128:(k + 1) * 128, :])
                            xcT_t.append(t)
                        w_tiles = {}
                        for wname, wext in (("wq", wq), ("wk", wk), ("wv", wv)):
                            tl = []
                            for k in range(NDD):
                                t = xw.tile([128, JC], F16, tag=f"{wname}{k}", name=f"{wname}{k}")
                                nc.sync.dma_start(t[:], wext[k * 128:(k + 1) * 128, :])
                                tl.append(t)
                            w_tiles[wname] = tl

                        vT = [vt.tile([128, LL], F16, tag=f"vT{h}", name=f"vT{h}") for h in range(NH)]

                        for h in range(NH):
                            for wname, outs in (("wq", qT), ("wk", kT), ("wv", vT)):
                                wt = w_tiles[wname]
                                for n in range(NLC):
                                    ps = pps.tile([128, 512], F32, tag="projps", name="projps")
                                    for k in range(NDD):
                                        nc.tensor.matmul(
                                            ps[:],
                                            wt[k][:, h * 128:(h + 1) * 128],
                                            xcT_t[k][:, n * 512:(n + 1) * 512],
                                            start=(k == 0), stop=(k == NDD - 1),
                                        )
                                    nc.vector.tensor_copy(
                                        outs[h][:, n * 512:(n + 1) * 512], ps[:])
                            # V^T[h] -> V[h] (seq-major) via PE 128x128
                            # transposes, 8 per PSUM bank + one batched copy
                            for g in range(NKB // 8):
                                tp = vtps.tile([128, 1024], F16, tag="vtp", name="vtp")
                                for j in range(8):
                                    kb = g * 8 + j
                                    nc.tensor.transpose(
                                        tp[:, j * 128:(j + 1) * 128],
                                        vT[h][:, kb * 128:(kb + 1) * 128], ident_t[:])
                                nc.vector.tensor_copy(
                                    vv[h][:, g * 1024:(g + 1) * 1024], tp[:])

                    if "B" in phases:
                        # ---------------- Phase B: flash attention per (head, q-chunk)
                        with (
                            tc.tile_pool(name="sps", bufs=4, space="PSUM") as sps,
                            tc.tile_pool(name="ops", bufs=2, space="PSUM") as ops,
                            tc.tile_pool(name="sums", bufs=2, space="PSUM") as sums,
                            tc.tile_pool(name="wpool", bufs=8) as wpool,
                            tc.tile_pool(name="bpool", bufs=2) as bpool,
                            tc.tile_pool(name="opool", bufs=2) as opool,
                        ):
                            for h in range(NH):
                                for qc in range(NLC):
                                    o_ps = ops.tile([128, 512], F32, tag="o", name="o")
                                    s_sum = sums.tile([1, 512], F32, tag="s", name="s")
                                    nkb = 4 * (qc + 1)
                                    # 1-stage software pipeline: issue the
                                    # next k-block's scores matmul before the
                                    # current block's ones/O matmuls so the PE
                                    # overlaps the exp on ScalarE.
                                    def consume(kb, f0, w, wx):
                                        nc.tensor.matmul(
                                            s_sum[0:1, f0:512], ones_t[:],
                                            wx[:, :w],
                                            start=(kb == 0), stop=(kb == nkb - 1))
                                        nc.tensor.matmul(
                                            o_ps[:, f0:512],
                                            vv[h][:, kb * 128:(kb + 1) * 128],
                                            wx[:, :w],
                                            start=(kb == 0), stop=(kb == nkb - 1))
                                    pend = None
                                    for kb in range(nkb):
                                        r = kb - 4 * qc
                                        f0 = 128 * r if r >= 0 else 0
                                        w = 512 - f0
                                        s_ps = sps.tile([128, 512], F32, tag="sc", name="sc")
                                        nc.tensor.matmul(
                                            s_ps[:, :w],
                                            kT[h][:, kb * 128:(kb + 1) * 128],
                                            qT[h][:, qc * 512 + f0: (qc + 1) * 512],
                                            start=True, stop=True)
                                        wx = wpool.tile([128, 512], F16, tag="wx", name="wx")
                                        nc.scalar.activation(
                                            wx[:, :w], s_ps[:, :w],
                                            mybir.ActivationFunctionType.Exp, scale=SCALE)
                                        if r >= 0:
                                            nc.vector.tensor_mul(
                                                wx[:, :128], wx[:, :128], mask_t[:])
                                        if pend is not None:
                                            consume(*pend)
                                        pend = (kb, f0, w, wx)
                                    consume(*pend)
                                    rec = bpool.tile([1, 512], F32, tag="rec", name="rec")
                                    nc.vector.reciprocal(rec[:], s_sum[:])
                                    bca = bpool.tile([128, 512], F32, tag="bca", name="bca")
                                    nc.gpsimd.partition_broadcast(bca[:], rec[:])
                                    osc = opool.tile([128, 512], F16, tag="osc", name="osc")
                                    nc.vector.tensor_mul(osc[:], o_ps[:], bca[:])
                                    nc.sync.dma_start(
                                        ag_in[h * 128:(h + 1) * 128,
                                              qc * 512:(qc + 1) * 512], osc[:])

                # ---------------- AllGather within each batch group --------------
                if with_cc and "C" in phases:
                    nc.gpsimd.collective_compute(
                        "AllGather",
                        mybir.AluOpType.bypass,
                        replica_groups=[[0, 1, 2, 3], [4, 5, 6, 7]],
                        ins=[ag_in.opt()],
                        outs=[ag_out.opt()],
                    )

                if "C" in phases:
                    # ---------------- Phase C: output projection ---------------------
                    with (
                        tc.tile_pool(name="ocp", bufs=1) as ocp,
                        tc.tile_pool(name="wop", bufs=1) as wop,
                        tc.tile_pool(name="yps", bufs=2, space="PSUM") as yps,
                        tc.tile_pool(name="ysb", bufs=3) as ysbp,
                    ):
                        oc_t = []
                        for k in range(NDD):
                            t = ocp.tile([128, LL], F16, tag=f"oc{k}", name=f"oc{k}")
                            nc.sync.dma_start(t[:], ag_out[k * 128:(k + 1) * 128, :])
                            oc_t.append(t)
                        wo_t = []
                        for k in range(NDD):
                            t = wop.tile([128, JC], F16, tag=f"wo{k}", name=f"wo{k}")
                            nc.sync.dma_start(t[:], wo[k * 128:(k + 1) * 128, :])
                            wo_t.append(t)

                        for m in range(NH):
                            pss = [yps.tile([128, 512], F32, tag=f"y{n}", name=f"y{n}")
                                   for n in range(NLC)]
                            for k in range(NDD):
                                for n in range(NLC):
                                    nc.tensor.matmul(
                                        pss[n][:],
                                        wo_t[k][:, m * 128:(m + 1) * 128],
                                        oc_t[k][:, n * 512:(n + 1) * 512],
                                        start=(k == 0), stop=(k == NDD - 1))
                            for n in range(NLC):
                                ysb = ysbp.tile([128, 512], F32, tag="ysb", name="ysb")
                                nc.scalar.activation(
                                    ysb[:], pss[n][:],
                                    mybir.ActivationFunctionType.Identity,
                                    bias=bo_t[:, m:m + 1], scale=1.0)
                                nc.sync.dma_start(
                                    yT[m * 128:(m + 1) * 128, n * 512:(n + 1) * 512],
                                    ysb[:])

    if compile:
        nc.compile()
    return nc


def _get(seq_len=L, repeat=1, with_cc=True):
    key = (seq_len, repeat, with_cc)
    if key not in _CACHE:
        _CACHE[key] = _build(seq_len, repeat=repeat, with_cc=with_cc)
    return _CACHE[key]


def _prep_inputs(x_real, x_imag, wq_r, wq_i, wk_r, wk_i, wv_r, wv_i,
                 wo_r, wo_i, bo_r, bo_i):
    """Host-side sharding: per-core input maps (fp16 layout prep)."""
    f16 = np.float16
    seq_len = x_real.shape[1]

    xcT_b = []
    for b in range(B):
        xcT_b.append(np.ascontiguousarray(
            np.concatenate([x_real[b].T, x_imag[b].T], axis=0)).astype(f16))

    mask01 = np.triu(np.ones((128, 128), dtype=np.float32)).astype(f16)
    ident = np.eye(128, dtype=np.float32).astype(f16)
    ones = np.ones((128, 1), dtype=f16)

    def proj_eff(w_r, w_i, heads):
        """[DD, 128*len(heads)] fp16: per head [r-cols(64) | i-cols(64)]."""
        w_eff = np.empty((DD, 128 * len(heads)), dtype=np.float32)
        for t, h in enumerate(heads):
            c0 = t * 128
            wr = w_r[64 * h:64 * h + 64, :].T    # [D, 64]
            wi = w_i[64 * h:64 * h + 64, :].T
            w_eff[:D, c0:c0 + 64] = wr
            w_eff[D:, c0:c0 + 64] = -wi
            w_eff[:D, c0 + 64:c0 + 128] = wi
            w_eff[D:, c0 + 64:c0 + 128] = wr
        return w_eff.astype(f16)

    in_maps = []
    for c in range(NCORES):
        b, g = divmod(c, GROUP)
        heads = [4 * g + t for t in range(NH)]
        ycols = slice(256 * g, 256 * g + 256)

        wq_eff = proj_eff(wq_r, wq_i, heads)
        wk_eff = proj_eff(wk_r, wk_i, heads)
        wv_eff = proj_eff(wv_r, wv_i, heads)

        # wo_eff rows follow the AllGather row order: for each global head hh:
        # 64 rows of o_r dims, then 64 rows of o_i dims.
        wo_eff = np.empty((DD, JC), dtype=np.float32)
        for hh in range(H):
            dr = slice(64 * hh, 64 * hh + 64)
            r0 = 128 * hh
            wo_eff[r0:r0 + 64, 0:256] = wo_r[ycols, dr].T
            wo_eff[r0 + 64:r0 + 128, 0:256] = -wo_i[ycols, dr].T
            wo_eff[r0:r0 + 64, 256:512] = wo_i[ycols, dr].T
            wo_eff[r0 + 64:r0 + 128, 256:512] = wo_r[ycols, dr].T
        wo_eff = wo_eff.astype(f16)

        bo_eff = np.concatenate(
            [bo_r[ycols], bo_i[ycols]]).astype(np.float32).reshape(JC, 1)

        in_maps.append({
            "xcT": xcT_b[b], "wq": wq_eff, "wk": wk_eff, "wv": wv_eff,
            "wo": wo_eff, "bo": bo_eff, "mask": mask01, "ident": ident,
            "ones": ones,
        })
    return in_maps, seq_len


def _run(in_maps, seq_len):
    nc = _get(seq_len)
    res = run_bass_kernel_spmd(nc, in_maps, core_ids=list(range(NCORES)),
                               trace=False)
    return res


def _assemble(results, seq_len):
    yr = np.empty((B, seq_len, D), dtype=np.float32)
    yi = np.empty((B, seq_len, D), dtype=np.float32)
    for c in range(NCORES):
        b, g = divmod(c, GROUP)
        yT_c = results[c]["yT"]                      # [512, LL]
        yr[b][:, 256 * g:256 * g + 256] = yT_c[:256].T
        yi[b][:, 256 * g:256 * g + 256] = yT_c[256:].T
    return yr, yi


def kernel(x_real, x_imag, wq_r, wq_i, wk_r, wk_i, wv_r, wv_i,
           wo_r, wo_i, bo_r, bo_i):
    args = [np.asarray(a) for a in (x_real, x_imag, wq_r, wq_i, wk_r, wk_i,
                                    wv_r, wv_i, wo_r, wo_i, bo_r, bo_i)]
    in_maps, seq_len = _prep_inputs(*args)
    res = _run(in_maps, seq_len)
    return _assemble(res.results, seq_len)



# revision 9
# speedup vs baseline: 1.0725x; 1.0725x over previous
"""Complex-valued causal attention on 8 trn2 NeuronCores.

nn_ComplexAttention: B=2, L=2048, D=1024, H=16 heads (hd=64), fp32 I/O.

Sharding: core c owns batch b = c//4 and heads 4*(c%4) .. 4*(c%4)+3.
Data parallel over B, tensor parallel over heads within a group; the 4
cores of a group AllGather the attention outputs (fp16) and each computes
a 256-column slice of the output projection.

Complex linears use Karatsuba (3 real matmuls instead of 4):
  m1 = x_r W_r^T, m2 = x_i W_i^T, m3 = (x_r+x_i)(W_r+W_i)^T
  y_r = m1 - m2 ; y_i = m3 - m1 - m2
Heads are processed in pairs so each m-chain runs with a full M=128
stationary tile; the pair-layout PSUM results are combined on DVE and
redistributed to per-head [r;i] layout with small SBUF->SBUF DMAs.

All on-chip math is fp16 with fp32 PSUM accumulation.  Everything is
d-major (transposed) so no activation transposes are needed; V is
re-laid seq-major with PE 128x128 transposes for the PV matmul.
"""

import sys

if "/opt/trn_rl_repo" not in sys.path:
    sys.path.insert(0, "/opt/trn_rl_repo")

import numpy as np

import concourse.mybir as mybir
import concourse.tile as tile
from concourse import bacc
from concourse.bass_utils import run_bass_kernel_spmd

B, L, D, H = 2, 2048, 1024, 16
HD = D // H            # 64
SCALE = HD ** (-0.5)
NCORES = 8
GROUP = 4              # cores per batch group
NH = H // GROUP        # 4 local heads per core
NPAIR = NH // 2        # head pairs per core
JC = NH * 2 * HD       # 512 local projection cols
D3 = 3 * D             # r rows ; i rows ; sum rows
F16 = mybir.dt.float16
F32 = mybir.dt.float32
ADD = mybir.AluOpType.add
SUB = mybir.AluOpType.subtract

_CACHE = {}


def _build(seq_len=L, repeat=1, with_cc=True, phases="ABC", compile=True):
    """Build + compile the SPMD kernel (identical program on all 8 cores)."""
    from contextlib import nullcontext
    LL = seq_len
    NLC = LL // 512        # l-chunks of 512
    NKB = LL // 128        # k-blocks of 128
    NK = D // 128          # contraction chunks per m-term (8)

    nc = bacc.Bacc("TRN2", target_bir_lowering=False, debug=False,
                   num_devices=NCORES)

    xr = nc.dram_tensor("xr", [D, LL], F16, kind="ExternalInput")
    xi = nc.dram_tensor("xi", [D, LL], F16, kind="ExternalInput")
    wq = nc.dram_tensor("wq", [D3, 256], F16, kind="ExternalInput")
    wk = nc.dram_tensor("wk", [D3, 256], F16, kind="ExternalInput")
    wv = nc.dram_tensor("wv", [D3, 256], F16, kind="ExternalInput")
    wo = nc.dram_tensor("wo", [D3, 256], F16, kind="ExternalInput")
    bo = nc.dram_tensor("bo", [JC, 1], F32, kind="ExternalInput")
    mask = nc.dram_tensor("mask", [128, 128], F16, kind="ExternalInput")
    ident = nc.dram_tensor("ident", [128, 128], F16, kind="ExternalInput")
    ones = nc.dram_tensor("ones", [128, 1], F16, kind="ExternalInput")
    yT = nc.dram_tensor("yT", [JC, LL], F16, kind="ExternalOutput")

    with tile.TileContext(nc) as tc:
        with (
            tc.tile_pool(name="const", bufs=1) as const,
            tc.tile_pool(name="dram", bufs=1, space="DRAM") as dram,
        ):
            mask_t = const.tile([128, 128], F16, tag="mask", name="mask")
            nc.sync.dma_start(mask_t[:], mask[:])
            ident_t = const.tile([128, 128], F16, tag="ident", name="ident")
            nc.sync.dma_start(ident_t[:], ident[:])
            ones_t = const.tile([128, 1], F16, tag="ones", name="ones")
            nc.sync.dma_start(ones_t[:], ones[:])
            bo_t = const.tile([128, NH], F32, tag="bo", name="bo")
            for m in range(NH):
                nc.sync.dma_start(bo_t[:, m:m + 1], bo[m * 128:(m + 1) * 128, :])

            ag_in = dram.tile([JC, LL], F16, tag="ag_in", name="ag_in")
            ag_out = dram.tile([2 * D, LL], F16, tag="ag_out", name="ag_out")

            _hint = (mybir.EngineType.PE, mybir.EngineType.Activation,
                     mybir.EngineType.DVE, mybir.EngineType.SP,
                     mybir.EngineType.Pool)
            loop_cm = (tc.For_i(0, repeat, 1, hint_engines=_hint)
                       if repeat > 1 else nullcontext())
            with loop_cm:
                with (
                    tc.tile_pool(name="qkvout", bufs=1) as qkvout,
                    tc.tile_pool(name="vpool", bufs=1) as vpool,
                    tc.tile_pool(name="wcpool", bufs=1) as wcpool,
                ):
                    qT = [qkvout.tile([128, LL], F16, tag=f"qT{h}", name=f"qT{h}") for h in range(NH)]
                    kT = [qkvout.tile([128, LL], F16, tag=f"kT{h}", name=f"kT{h}") for h in range(NH)]
                    vv = [vpool.tile([128, LL], F16, tag=f"v{h}", name=f"v{h}") for h in range(NH)]

                    # ---------------- Phase A: projections + V transpose ----------
                    with (
                        tc.tile_pool(name="wproj", bufs=1) as wproj,
                        tc.tile_pool(name="xpool", bufs=2) as xpool,
                        tc.tile_pool(name="vt", bufs=2) as vt,
                        tc.tile_pool(name="comb", bufs=4) as compool,
                        tc.tile_pool(name="pps", bufs=2, space="PSUM") as pps,
                        tc.tile_pool(name="vtps", bufs=2, space="PSUM") as vtps,
                    ):
                        # wq first so the first Q chain can start ASAP; wk/wv
                        # are issued behind the n=0 x block (see below).
                        w_tiles = {}
                        for wname, wext in (("wq", wq), ("wk", wk), ("wv", wv)):
                            tl = []
                            for k in range(3 * NK):
                                t = wproj.tile([128, 256], F16,
                                               tag=f"{wname}{k}", name=f"{wname}{k}")
                                if wname == "wq":
                                    nc.sync.dma_start(t[:], wext[k * 128:(k + 1) * 128, :])
                                tl.append(t)
                            w_tiles[wname] = tl

                        for n in range(NLC):
                            # x column block for this n-chunk
                            xr_t, xi_t, xs_t = [], [], []
                            for k in range(NK):
                                t = xpool.tile([128, 512], F16, tag=f"xr{k}", name=f"xr{k}")
                                nc.sync.dma_start(t[:], xr[k * 128:(k + 1) * 128,
                                                           n * 512:(n + 1) * 512])
                                xr_t.append(t)
                            for k in range(NK):
                                t = xpool.tile([128, 512], F16, tag=f"xi{k}", name=f"xi{k}")
                                nc.sync.dma_start(t[:], xi[k * 128:(k + 1) * 128,
                                                           n * 512:(n + 1) * 512])
                                xi_t.append(t)
                            for k in range(NK):
                                t = xpool.tile([128, 512], F16, tag=f"xs{k}", name=f"xs{k}")
                                nc.gpsimd.tensor_tensor(t[:], xr_t[k][:], xi_t[k][:], ADD)
                                xs_t.append(t)
                            if n == 0:
                                for wname, wext in (("wk", wk), ("wv", wv)):
                                    for k in range(3 * NK):
                                        nc.sync.dma_start(
                                            w_tiles[wname][k][:],
                                            wext[k * 128:(k + 1) * 128, :])

                            vT_n = [vt.tile([128, 512], F16, tag=f"vT{h}", name=f"vT{h}")
                                    for h in range(NH)]

                            for wname, wext in (("wq", qT), ("wk", kT), ("wv", None)):
                                wt = w_tiles[wname]
                                for p in range(NPAIR):
                                    cs = slice(128 * p, 128 * p + 128)
                                    psA = pps.tile([128, 512], F32, tag="psA", name="psA")
                                    for k in range(NK):
                                        nc.tensor.matmul(
                                            psA[:], wt[k][:, cs], xr_t[k][:],
                                            start=(k == 0), stop=(k == NK - 1))
                                    psB = pps.tile([128, 512], F32, tag="psB", name="psB")
                                    for k in range(NK):
                                        nc.tensor.matmul(
                                            psB[:], wt[NK + k][:, cs], xi_t[k][:],
                                            start=(k == 0), stop=(k == NK - 1))
                                    psC = pps.tile([128, 512], F32, tag="psC", name="psC")
                                    for k in range(NK):
                                        nc.tensor.matmul(
                                            psC[:], wt[2 * NK + k][:, cs], xs_t[k][:],
                                            start=(k == 0), stop=(k == NK - 1))
                                    # pair-layout combines: stage m1 to SBUF on
                                    # ScalarE (DVE can read only one PSUM input)
                                    sA = compool.tile([128, 512], F32, tag="sA", name="sA")
                                    nc.scalar.activation(
                                        sA[:], psA[:],
                                        mybir.ActivationFunctionType.Identity)
                                    rP = compool.tile([128, 512], F16, tag="rP", name="rP")
                                    nc.vector.tensor_tensor(rP[:], sA[:], psB[:], SUB)
                                    tm = compool.tile([128, 512], F32, tag="tm", name="tm")
                                    nc.vector.tensor_tensor(tm[:], sA[:], psB[:], ADD)
                                    iP = compool.tile([128, 512], F16, tag="iP", name="iP")
                                    nc.vector.tensor_tensor(iP[:], psC[:], tm[:], SUB)
                                    # redistribute to per-head [r;i] tiles
                                    for t in range(2):
                                        h = 2 * p + t
                                        dst = (wext[h] if wext is not None else vT_n[h])
                                        col = (slice(n * 512, n * 512 + 512)
                                               if wext is not None else slice(0, 512))
                                        nc.sync.dma_start(
                                            dst[0:64, col], rP[64 * t:64 * t + 64, :])
                                        nc.sync.dma_start(
                                            dst[64:128, col], iP[64 * t:64 * t + 64, :])

                            # V -> seq-major via PE transposes (4 kb per n-chunk)
                            for h in range(NH):
                                tp = vtps.tile([128, 512], F16, tag="vtp", name="vtp")
                                for j in range(4):
                                    nc.tensor.transpose(
                                        tp[:, j * 128:(j + 1) * 128],
                                        vT_n[h][:, j * 128:(j + 1) * 128], ident_t[:])
                                nc.vector.tensor_copy(
                                    vv[h][:, n * 512:(n + 1) * 512], tp[:])

                    # C-phase weights: DMA during B (engines idle)
                    wo_t = []
                    for k in range(3 * NK):
                        t = wcpool.tile([128, 256], F16, tag=f"wo{k}", name=f"wo{k}")
                        nc.sync.dma_start(t[:], wo[k * 128:(k + 1) * 128, :])
                        wo_t.append(t)

                    if "B" in phases:
                        # ---------------- Phase B: flash attention per (head, q-chunk)
                        with (
                            tc.tile_pool(name="sps", bufs=4, space="PSUM") as sps,
                            tc.tile_pool(name="ops", bufs=2, space="PSUM") as ops,
                            tc.tile_pool(name="sums", bufs=2, space="PSUM") as sums,
                            tc.tile_pool(name="wpool", bufs=8) as wpool,
                            tc.tile_pool(name="bpool", bufs=2) as bpool,
                            tc.tile_pool(name="opool", bufs=2) as opool,
                        ):
                            for h in range(NH):
                                for qc in range(NLC):
                                    o_ps = ops.tile([128, 512], F32, tag="o", name="o")
                                    s_sum = sums.tile([1, 512], F32, tag="s", name="s")
                                    nkb = 4 * (qc + 1)
                                    # 1-stage software pipeline: issue the
                                    # next k-block's scores matmul before the
                                    # current block's ones/O matmuls so the PE
                                    # overlaps the exp on ScalarE.
                                    def consume(kb, f0, w, wx):
                                        nc.tensor.matmul(
                                            s_sum[0:1, f0:512], ones_t[:],
                                            wx[:, :w],
                                            start=(kb == 0), stop=(kb == nkb - 1))
                                        nc.tensor.matmul(
                                            o_ps[:, f0:512],
                                            vv[h][:, kb * 128:(kb + 1) * 128],
                                            wx[:, :w],
                                            start=(kb == 0), stop=(kb == nkb - 1))
                                    pend = None
                                    for kb in range(nkb):
                                        r = kb - 4 * qc
                                        f0 = 128 * r if r >= 0 else 0
                                        w = 512 - f0
                                        s_ps = sps.tile([128, 512], F32, tag="sc", name="sc")
                                        nc.tensor.matmul(
                                            s_ps[:, :w],
                                            kT[h][:, kb * 128:(kb + 1) * 128],
                                            qT[h][:, qc * 512 + f0: (qc + 1) * 512],
                                            start=True, stop=True)
                                        wx = wpool.tile([128, 512], F16, tag="wx", name="wx")
                                        nc.scalar.activation(
                                            wx[:, :w], s_ps[:, :w],
                                            mybir.ActivationFunctionType.Exp, scale=SCALE)
                                        if r >= 0:
                                            nc.vector.tensor_mul(
                                                wx[:, :128], wx[:, :128], mask_t[:])
                                        if pend is not None:
                                            consume(*pend)
                                        pend = (kb, f0, w, wx)
                                    consume(*pend)
                                    rec = bpool.tile([1, 512], F32, tag="rec", name="rec")
                                    nc.vector.reciprocal(rec[:], s_sum[:])
                                    bca = bpool.tile([128, 512], F32, tag="bca", name="bca")
                                    nc.gpsimd.partition_broadcast(bca[:], rec[:])
                                    osc = opool.tile([128, 512], F16, tag="osc", name="osc")
                                    nc.vector.tensor_mul(osc[:], o_ps[:], bca[:])
                                    # r-part rows then i-part rows of the group layout
                                    nc.sync.dma_start(
                                        ag_in[h * 64:(h + 1) * 64,
                                              qc * 512:(qc + 1) * 512], osc[0:64, :])
                                    nc.sync.dma_start(
                                        ag_in[256 + h * 64:256 + (h + 1) * 64,
                                              qc * 512:(qc + 1) * 512], osc[64:128, :])

                    # ---------------- AllGather within each batch group --------------
                    if with_cc and "C" in phases:
                        nc.gpsimd.collective_compute(
                            "AllGather",
                            mybir.AluOpType.bypass,
                            replica_groups=[[0, 1, 2, 3], [4, 5, 6, 7]],
                            ins=[ag_in.opt()],
                            outs=[ag_out.opt()],
                        )

                    if "C" in phases:
                        # ---------------- Phase C: output projection -----------------
                        # ag_out rows: per rank g: [4*64 r-rows ; 4*64 i-rows]
                        # oc tile ki=4g+j: j in 0,1 -> r-chunk c=2g+j ; j in 2,3 ->
                        # i-chunk c=2g+(j-2).  wo rows: 0:1024 r (chunk c at 128c),
                        # 1024:2048 i, 2048:3072 sum.
                        with (
                            tc.tile_pool(name="ocp", bufs=1) as ocp,
                            tc.tile_pool(name="yps", bufs=2, space="PSUM") as yps,
                            tc.tile_pool(name="ysb", bufs=4) as ysbp,
                        ):
                            oc_r, oc_i, oc_s = [], [], []
                            for c in range(NK):
                                g, j = divmod(c, 2)
                                ki = 4 * g + j
                                t = ocp.tile([128, LL], F16, tag=f"ocr{c}", name=f"ocr{c}")
                                nc.sync.dma_start(t[:], ag_out[ki * 128:(ki + 1) * 128, :])
                                oc_r.append(t)
                            for c in range(NK):
                                g, j = divmod(c, 2)
                                ki = 4 * g + 2 + j
                                t = ocp.tile([128, LL], F16, tag=f"oci{c}", name=f"oci{c}")
                                nc.sync.dma_start(t[:], ag_out[ki * 128:(ki + 1) * 128, :])
                                oc_i.append(t)
                            for c in range(NK):
                                t = ocp.tile([128, LL], F16, tag=f"ocs{c}", name=f"ocs{c}")
                                nc.gpsimd.tensor_tensor(t[:], oc_r[c][:], oc_i[c][:], ADD)
                                oc_s.append(t)

                            for n in range(NLC):
                                col = slice(n * 512, (n + 1) * 512)
                                for g in range(2):
                                    cs = slice(128 * g, 128 * g + 128)
                                    m1 = yps.tile([128, 512], F32, tag="m1", name="m1")
                                    for k in range(NK):
                                        nc.tensor.matmul(
                                            m1[:], wo_t[k][:, cs], oc_r[k][:, col],
                                            start=(k == 0), stop=(k == NK - 1))
                                    m2 = yps.tile([128, 512], F32, tag="m2", name="m2")
                                    for k in range(NK):
                                        nc.tensor.matmul(
                                            m2[:], wo_t[NK + k][:, cs],
                                            oc_i[k][:, col],
                                            start=(k == 0), stop=(k == NK - 1))
                                    m3 = yps.tile([128, 512], F32, tag="m3", name="m3")
                                    for k in range(NK):
                                        nc.tensor.matmul(
                                            m3[:], wo_t[2 * NK + k][:, cs], oc_s[k][:, col],
                                            start=(k == 0), stop=(k == NK - 1))
                                    s1 = ysbp.tile([128, 512], F32, tag="s1", name="s1")
                                    nc.scalar.activation(
                                        s1[:], m1[:],
                                        mybir.ActivationFunctionType.Identity)
                                    yr = ysbp.tile([128, 512], F16, tag="yr", name="yr")
                                    nc.vector.scalar_tensor_tensor(
                                        yr[:], s1[:], bo_t[:, g:g + 1], m2[:],
                                        op0=ADD, op1=SUB)
                                    nc.sync.dma_start(yT[g * 128:(g + 1) * 128, col], yr[:])
                                    tm = ysbp.tile([128, 512], F32, tag="ytm", name="ytm")
                                    nc.vector.tensor_tensor(tm[:], s1[:], m2[:], ADD)
                                    yi = ysbp.tile([128, 512], F16, tag="yi", name="yi")
                                    nc.vector.scalar_tensor_tensor(
                                        yi[:], m3[:], bo_t[:, 2 + g:3 + g], tm[:],
                                        op0=ADD, op1=SUB)
                                    nc.sync.dma_start(
                                        yT[256 + g * 128:256 + (g + 1) * 128, col], yi[:])

    if compile:
        nc.compile()
    return nc


def _get(seq_len=L, repeat=1, with_cc=True):
    key = (seq_len, repeat, with_cc)
    if key not in _CACHE:
        _CACHE[key] = _build(seq_len, repeat=repeat, with_cc=with_cc)
    return _CACHE[key]


def _prep_inputs(x_real, x_imag, wq_r, wq_i, wk_r, wk_i, wv_r, wv_i,
                 wo_r, wo_i, bo_r, bo_i):
    """Host-side sharding: per-core input maps (fp16 layout prep)."""
    f16 = np.float16
    seq_len = x_real.shape[1]

    xr_b, xi_b = [], []
    for b in range(B):
        xr_b.append(np.ascontiguousarray(x_real[b].T).astype(f16))
        xi_b.append(np.ascontiguousarray(x_imag[b].T).astype(f16))

    mask01 = np.triu(np.ones((128, 128), dtype=np.float32)).astype(f16)
    ident = np.eye(128, dtype=np.float32).astype(f16)
    ones = np.ones((128, 1), dtype=f16)

    def proj_eff(w_r, w_i, heads):
        """Karatsuba weights [3D, 256]: rows (Wr ; Wi ; Wr+Wi), cols by
        head pair [h0 | h1] then [h2 | h3]."""
        w_eff = np.empty((D3, 256), dtype=np.float32)
        w_s = w_r + w_i
        for t, h in enumerate(heads):
            c0 = 64 * t
            rows = slice(64 * h, 64 * h + 64)
            w_eff[:D, c0:c0 + 64] = w_r[rows, :].T
            w_eff[D:2 * D, c0:c0 + 64] = w_i[rows, :].T
            w_eff[2 * D:, c0:c0 + 64] = w_s[rows, :].T
        return w_eff.astype(f16)

    in_maps = []
    for c in range(NCORES):
        b, g = divmod(c, GROUP)
        heads = [4 * g + t for t in range(NH)]
        ycols = slice(256 * g, 256 * g + 256)

        wq_eff = proj_eff(wq_r, wq_i, heads)
        wk_eff = proj_eff(wk_r, wk_i, heads)
        wv_eff = proj_eff(wv_r, wv_i, heads)

        # out-proj Karatsuba weights [3D, 256]: contraction rows follow the
        # AllGather layout remapped to (all-r ; all-i ; all-sum) chunks.
        wo_eff = np.empty((D3, 256), dtype=np.float32)
        wo_s = wo_r + wo_i
        wo_eff[:D, :] = wo_r[ycols, :].T
        wo_eff[D:2 * D, :] = wo_i[ycols, :].T
        wo_eff[2 * D:, :] = wo_s[ycols, :].T
        wo_eff = wo_eff.astype(f16)

        bo_eff = np.concatenate(
            [bo_r[ycols], bo_i[ycols]]).astype(np.float32).reshape(JC, 1)

        in_maps.append({
            "xr": xr_b[b], "xi": xi_b[b], "wq": wq_eff, "wk": wk_eff,
            "wv": wv_eff, "wo": wo_eff, "bo": bo_eff, "mask": mask01,
            "ident": ident, "ones": ones,
        })
    return in_maps, seq_len


def _run(in_maps, seq_len):
    nc = _get(seq_len)
    res = run_bass_kernel_spmd(nc, in_maps, core_ids=list(range(NCORES)),
                               trace=False)
    return res


def _assemble(results, seq_len):
    yr = np.empty((B, seq_len, D), dtype=np.float32)
    yi = np.empty((B, seq_len, D), dtype=np.float32)
    for c in range(NCORES):
        b, g = divmod(c, GROUP)
        yT_c = results[c]["yT"]                      # [512, LL] fp16
        yr[b][:, 256 * g:256 * g + 256] = yT_c[:256].T.astype(np.float32)
        yi[b][:, 256 * g:256 * g + 256] = yT_c[256:].T.astype(np.float32)
    return yr, yi


def kernel(x_real, x_imag, wq_r, wq_i, wk_r, wk_i, wv_r, wv_i,
           wo_r, wo_i, bo_r, bo_i):
    args = [np.asarray(a) for a in (x_real, x_imag, wq_r, wq_i, wk_r, wk_i,
                                    wv_r, wv_i, wo_r, wo_i, bo_r, bo_i)]
    in_maps, seq_len = _prep_inputs(*args)
    res = _run(in_maps, seq_len)
    return _assemble(res.results, seq_len)


# revision 12
# speedup vs baseline: 1.1784x; 1.0987x over previous
"""Complex-valued causal attention on 8 trn2 NeuronCores.

nn_ComplexAttention: B=2, L=2048, D=1024, H=16 heads (hd=64), fp32 I/O.

Sharding: core c owns batch b = c//4 and heads 4*(c%4) .. 4*(c%4)+3.
Data parallel over B, tensor parallel over heads within a group; the 4
cores of a group AllGather the attention outputs (fp16) and each computes
a 256-column slice of the output projection.

Complex linears use Karatsuba (3 real matmuls instead of 4):
  m1 = x_r W_r^T, m2 = x_i W_i^T, m3 = (x_r+x_i)(W_r+W_i)^T
  y_r = m1 - m2 ; y_i = m3 - m1 - m2
Heads are processed in pairs so each m-chain runs with a full M=128
stationary tile; the pair-layout PSUM results are combined on DVE and
redistributed to per-head [r;i] layout with small SBUF->SBUF DMAs.

All on-chip math is fp16 with fp32 PSUM accumulation.  Everything is
d-major (transposed) so no activation transposes are needed; V is
re-laid seq-major with PE 128x128 transposes for the PV matmul.
"""

import sys

if "/opt/trn_rl_repo" not in sys.path:
    sys.path.insert(0, "/opt/trn_rl_repo")

import numpy as np

import concourse.mybir as mybir
import concourse.tile as tile
from concourse import bacc
from concourse.bass_utils import run_bass_kernel_spmd

B, L, D, H = 2, 2048, 1024, 16
HD = D // H            # 64
SCALE = HD ** (-0.5)
NCORES = 8
GROUP = 4              # cores per batch group
NH = H // GROUP        # 4 local heads per core
NPAIR = NH // 2        # head pairs per core
JC = NH * 2 * HD       # 512 local projection cols
D3 = 3 * D             # r rows ; i rows ; sum rows
F16 = mybir.dt.float16
F32 = mybir.dt.float32
ADD = mybir.AluOpType.add
SUB = mybir.AluOpType.subtract

_CACHE = {}


def _build(seq_len=L, repeat=1, with_cc=True, phases="ABC", compile=True):
    """Build + compile the SPMD kernel (identical program on all 8 cores)."""
    from contextlib import nullcontext
    LL = seq_len
    NLC = LL // 512        # l-chunks of 512
    NKB = LL // 128        # k-blocks of 128
    NK = D // 128          # contraction chunks per m-term (8)

    nc = bacc.Bacc("TRN2", target_bir_lowering=False, debug=False,
                   num_devices=NCORES)

    xr = nc.dram_tensor("xr", [D, LL], F16, kind="ExternalInput")
    xi = nc.dram_tensor("xi", [D, LL], F16, kind="ExternalInput")
    wq = nc.dram_tensor("wq", [D3, 256], F16, kind="ExternalInput")
    wk = nc.dram_tensor("wk", [D3, 256], F16, kind="ExternalInput")
    wv = nc.dram_tensor("wv", [D3, 256], F16, kind="ExternalInput")
    wo = nc.dram_tensor("wo", [D3, 256], F16, kind="ExternalInput")
    bo = nc.dram_tensor("bo", [JC, 1], F32, kind="ExternalInput")
    mask = nc.dram_tensor("mask", [128, 128], F16, kind="ExternalInput")
    ident = nc.dram_tensor("ident", [128, 128], F16, kind="ExternalInput")
    ones = nc.dram_tensor("ones", [128, 1], F16, kind="ExternalInput")
    yT = nc.dram_tensor("yT", [JC, LL], F16, kind="ExternalOutput")

    with tile.TileContext(nc) as tc:
        with (
            tc.tile_pool(name="const", bufs=1) as const,
            tc.tile_pool(name="dram", bufs=1, space="DRAM") as dram,
        ):
            mask_t = const.tile([128, 128], F16, tag="mask", name="mask")
            nc.sync.dma_start(mask_t[:], mask[:])
            ident_t = const.tile([128, 128], F16, tag="ident", name="ident")
            nc.sync.dma_start(ident_t[:], ident[:])
            ones_t = const.tile([128, 1], F16, tag="ones", name="ones")
            nc.sync.dma_start(ones_t[:], ones[:])
            bo_t = const.tile([128, NH], F32, tag="bo", name="bo")
            for m in range(NH):
                nc.sync.dma_start(bo_t[:, m:m + 1], bo[m * 128:(m + 1) * 128, :])

            ag_in = dram.tile([JC, LL], F16, tag="ag_in", name="ag_in")
            ag_out = dram.tile([2 * D, LL], F16, tag="ag_out", name="ag_out")

            _hint = (mybir.EngineType.PE, mybir.EngineType.Activation,
                     mybir.EngineType.DVE, mybir.EngineType.SP,
                     mybir.EngineType.Pool)
            loop_cm = (tc.For_i(0, repeat, 1, hint_engines=_hint)
                       if repeat > 1 else nullcontext())
            with loop_cm:
                with (
                    tc.tile_pool(name="qkvout", bufs=1) as qkvout,
                    tc.tile_pool(name="vpool", bufs=1) as vpool,
                    tc.tile_pool(name="wcpool", bufs=1) as wcpool,
                ):
                    qT = [qkvout.tile([128, LL], F16, tag=f"qT{h}", name=f"qT{h}") for h in range(NH)]
                    kT = [qkvout.tile([128, LL], F16, tag=f"kT{h}", name=f"kT{h}") for h in range(NH)]
                    vv = [vpool.tile([128, LL], F16, tag=f"v{h}", name=f"v{h}") for h in range(NH)]

                    # ---------------- Phase A: projections + V transpose ----------
                    with (
                        tc.tile_pool(name="wproj", bufs=1) as wproj,
                        tc.tile_pool(name="xpool", bufs=2) as xpool,
                        tc.tile_pool(name="vt", bufs=2) as vt,
                        tc.tile_pool(name="comb", bufs=4) as compool,
                        tc.tile_pool(name="pps", bufs=2, space="PSUM") as pps,
                        tc.tile_pool(name="vtps", bufs=2, space="PSUM") as vtps,
                    ):
                        # wq first so the first Q chain can start ASAP; wk/wv
                        # are issued behind the n=0 x block (see below).
                        w_tiles = {}
                        for wname, wext in (("wq", wq), ("wk", wk), ("wv", wv)):
                            tl = []
                            for k in range(3 * NK):
                                t = wproj.tile([128, 256], F16,
                                               tag=f"{wname}{k}", name=f"{wname}{k}")
                                if wname == "wq":
                                    nc.sync.dma_start(t[:], wext[k * 128:(k + 1) * 128, :])
                                tl.append(t)
                            w_tiles[wname] = tl

                        for n in range(NLC):
                            # x column block for this n-chunk
                            xr_t, xi_t, xs_t = [], [], []
                            for k in range(NK):
                                t = xpool.tile([128, 512], F16, tag=f"xr{k}", name=f"xr{k}")
                                nc.sync.dma_start(t[:], xr[k * 128:(k + 1) * 128,
                                                           n * 512:(n + 1) * 512])
                                xr_t.append(t)
                            for k in range(NK):
                                t = xpool.tile([128, 512], F16, tag=f"xi{k}", name=f"xi{k}")
                                nc.sync.dma_start(t[:], xi[k * 128:(k + 1) * 128,
                                                           n * 512:(n + 1) * 512])
                                xi_t.append(t)
                            for k in range(NK):
                                t = xpool.tile([128, 512], F16, tag=f"xs{k}", name=f"xs{k}")
                                nc.gpsimd.tensor_tensor(t[:], xr_t[k][:], xi_t[k][:], ADD)
                                xs_t.append(t)
                            if n == 0:
                                for wname, wext in (("wk", wk), ("wv", wv)):
                                    for k in range(3 * NK):
                                        nc.sync.dma_start(
                                            w_tiles[wname][k][:],
                                            wext[k * 128:(k + 1) * 128, :])

                            vT_n = [vt.tile([128, 512], F16, tag=f"vT{h}", name=f"vT{h}")
                                    for h in range(NH)]

                            for wname, wext in (("wq", qT), ("wk", kT), ("wv", None)):
                                wt = w_tiles[wname]
                                for p in range(NPAIR):
                                    cs = slice(128 * p, 128 * p + 128)
                                    psA = pps.tile([128, 512], F32, tag="psA", name="psA")
                                    for k in range(NK):
                                        nc.tensor.matmul(
                                            psA[:], wt[k][:, cs], xr_t[k][:],
                                            start=(k == 0), stop=(k == NK - 1))
                                    psB = pps.tile([128, 512], F32, tag="psB", name="psB")
                                    for k in range(NK):
                                        nc.tensor.matmul(
                                            psB[:], wt[NK + k][:, cs], xi_t[k][:],
                                            start=(k == 0), stop=(k == NK - 1))
                                    psC = pps.tile([128, 512], F32, tag="psC", name="psC")
                                    for k in range(NK):
                                        nc.tensor.matmul(
                                            psC[:], wt[2 * NK + k][:, cs], xs_t[k][:],
                                            start=(k == 0), stop=(k == NK - 1))
                                    # pair-layout combines: stage m1 to SBUF on
                                    # ScalarE (DVE can read only one PSUM input)
                                    sA = compool.tile([128, 512], F32, tag="sA", name="sA")
                                    nc.scalar.activation(
                                        sA[:], psA[:],
                                        mybir.ActivationFunctionType.Identity)
                                    rP = compool.tile([128, 512], F16, tag="rP", name="rP")
                                    nc.vector.tensor_tensor(rP[:], sA[:], psB[:], SUB)
                                    tm = compool.tile([128, 512], F32, tag="tm", name="tm")
                                    nc.vector.tensor_tensor(tm[:], sA[:], psB[:], ADD)
                                    iP = compool.tile([128, 512], F16, tag="iP", name="iP")
                                    nc.vector.tensor_tensor(iP[:], psC[:], tm[:], SUB)
                                    # redistribute to per-head [r;i] tiles
                                    for t in range(2):
                                        h = 2 * p + t
                                        dst = (wext[h] if wext is not None else vT_n[h])
                                        col = (slice(n * 512, n * 512 + 512)
                                               if wext is not None else slice(0, 512))
                                        nc.sync.dma_start(
                                            dst[0:64, col], rP[64 * t:64 * t + 64, :])
                                        nc.sync.dma_start(
                                            dst[64:128, col], iP[64 * t:64 * t + 64, :])

                            # V -> seq-major via PE transposes (4 kb per n-chunk)
                            for h in range(NH):
                                tp = vtps.tile([128, 512], F16, tag="vtp", name="vtp")
                                for j in range(4):
                                    nc.tensor.transpose(
                                        tp[:, j * 128:(j + 1) * 128],
                                        vT_n[h][:, j * 128:(j + 1) * 128], ident_t[:])
                                nc.vector.tensor_copy(
                                    vv[h][:, n * 512:(n + 1) * 512], tp[:])

                    # C-phase weights: DMA during B (engines idle)
                    wo_t = []
                    for k in range(3 * NK):
                        t = wcpool.tile([128, 256], F16, tag=f"wo{k}", name=f"wo{k}")
                        nc.sync.dma_start(t[:], wo[k * 128:(k + 1) * 128, :])
                        wo_t.append(t)

                    if "B" in phases:
                        # ---------------- Phase B: flash attention per (head, q-chunk)
                        # k-blocks in PAIRS: two score matmuls fill the halves
                        # of a [128,1024] PSUM tile (2 banks) and one exp
                        # covers both.  Softmax denominators come from a fp16
                        # wsum accumulated on DVE (replaces per-block
                        # ones-matmuls on PE); one ones-matmul per (h,qc)
                        # reduces wsum over partitions.
                        with (
                            tc.tile_pool(name="sps", bufs=2, space="PSUM") as sps,
                            tc.tile_pool(name="ops", bufs=2, space="PSUM") as ops,
                            tc.tile_pool(name="sums", bufs=2, space="PSUM") as sums,
                            tc.tile_pool(name="wpool", bufs=6) as wpool,
                            tc.tile_pool(name="wsump", bufs=2) as wsump,
                            tc.tile_pool(name="bpool", bufs=2) as bpool,
                            tc.tile_pool(name="opool", bufs=2) as opool,
                        ):
                            for h in range(NH):
                                for qc in range(NLC):
                                    o_ps = ops.tile([128, 512], F32, tag="o", name="o")
                                    s_sum = sums.tile([1, 512], F32, tag="s", name="s")
                                    wsum = wsump.tile([128, 512], F16, tag="ws", name="ws")
                                    nkb = 4 * (qc + 1)
                                    units = [("full", 2 * j) for j in range(2 * qc)]
                                    units += [("diag", 4 * qc), ("diag", 4 * qc + 2)]

                                    def produce(kind, kb0):
                                        s_t = sps.tile([128, 1024], F32, tag="sc", name="sc")
                                        wx = wpool.tile([128, 1024], F16, tag="wx", name="wx")
                                        if kind == "full":
                                            w0 = w1 = 512
                                            for s_ in range(2):
                                                nc.tensor.matmul(
                                                    s_t[:, s_ * 512:(s_ + 1) * 512],
                                                    kT[h][:, (kb0 + s_) * 128:(kb0 + s_ + 1) * 128],
                                                    qT[h][:, qc * 512:(qc + 1) * 512],
                                                    start=True, stop=True)
                                            nc.scalar.activation(
                                                wx[:], s_t[:],
                                                mybir.ActivationFunctionType.Exp,
                                                scale=SCALE)
                                        else:
                                            r0 = kb0 - 4 * qc
                                            w0, w1 = 512 - 128 * r0, 384 - 128 * r0
                                            nc.tensor.matmul(
                                                s_t[:, :w0],
                                                kT[h][:, kb0 * 128:(kb0 + 1) * 128],
                                                qT[h][:, qc * 512 + 128 * r0:(qc + 1) * 512],
                                                start=True, stop=True)
                                            nc.tensor.matmul(
                                                s_t[:, 512:512 + w1],
                                                kT[h][:, (kb0 + 1) * 128:(kb0 + 2) * 128],
                                                qT[h][:, qc * 512 + 128 * (r0 + 1):(qc + 1) * 512],
                                                start=True, stop=True)
                                            nc.scalar.activation(
                                                wx[:, :512 + w1], s_t[:, :512 + w1],
                                                mybir.ActivationFunctionType.Exp,
                                                scale=SCALE)
                                            nc.vector.tensor_mul(
                                                wx[:, :128], wx[:, :128], mask_t[:])
                                            nc.vector.tensor_mul(
                                                wx[:, 512:640], wx[:, 512:640], mask_t[:])
                                        return (kind, kb0, wx, w0, w1)

                                    def consume(u):
                                        kind, kb0, wx, w0, w1 = u
                                        o0 = 0 if kind == "full" else 512 - w0
                                        o1 = 0 if kind == "full" else 640 - w0
                                        nc.tensor.matmul(
                                            o_ps[:, o0:512],
                                            vv[h][:, kb0 * 128:(kb0 + 1) * 128],
                                            wx[:, :w0],
                                            start=(kb0 == 0), stop=False)
                                        nc.tensor.matmul(
                                            o_ps[:, o1:512],
                                            vv[h][:, (kb0 + 1) * 128:(kb0 + 2) * 128],
                                            wx[:, 512:512 + w1],
                                            start=False, stop=(kb0 + 2 == nkb))
                                        if kb0 == 0:
                                            if kind == "full":
                                                nc.vector.tensor_tensor(
                                                    wsum[:], wx[:, 0:512],
                                                    wx[:, 512:1024], ADD)
                                            else:
                                                nc.vector.tensor_copy(
                                                    wsum[:], wx[:, 0:512])
                                                nc.vector.tensor_add(
                                                    wsum[:, 128:], wsum[:, 128:],
                                                    wx[:, 512:512 + w1])
                                        else:
                                            nc.vector.tensor_add(
                                                wsum[:, o0:], wsum[:, o0:], wx[:, :w0])
                                            nc.vector.tensor_add(
                                                wsum[:, o1:], wsum[:, o1:],
                                                wx[:, 512:512 + w1])

                                    pend = None
                                    for u in units:
                                        uu = produce(*u)
                                        if pend is not None:
                                            consume(pend)
                                        pend = uu
                                    consume(pend)
                                    nc.tensor.matmul(s_sum[0:1, :], ones_t[:],
                                                     wsum[:], start=True, stop=True)
                                    rec = bpool.tile([1, 512], F32, tag="rec", name="rec")
                                    nc.vector.reciprocal_approx_fast(rec[:], s_sum[:])
                                    bca = bpool.tile([128, 512], F32, tag="bca", name="bca")
                                    nc.gpsimd.partition_broadcast(bca[:], rec[:])
                                    osc = opool.tile([128, 512], F16, tag="osc", name="osc")
                                    nc.vector.tensor_mul(osc[:], o_ps[:], bca[:])
                                    # r-part rows then i-part rows of the group layout
                                    nc.sync.dma_start(
                                        ag_in[h * 64:(h + 1) * 64,
                                              qc * 512:(qc + 1) * 512], osc[0:64, :])
                                    nc.sync.dma_start(
                                        ag_in[256 + h * 64:256 + (h + 1) * 64,
                                              qc * 512:(qc + 1) * 512], osc[64:128, :])

                    # ---------------- AllGather within each batch group --------------
                    if with_cc and "C" in phases:
                        nc.gpsimd.collective_compute(
                            "AllGather",
                            mybir.AluOpType.bypass,
                            replica_groups=[[0, 1, 2, 3], [4, 5, 6, 7]],
                            ins=[ag_in.opt()],
                            outs=[ag_out.opt()],
                        )

                    if "C" in phases:
                        # ---------------- Phase C: output projection -----------------
                        # ag_out rows: per rank g: [4*64 r-rows ; 4*64 i-rows]
                        # oc tile ki=4g+j: j in 0,1 -> r-chunk c=2g+j ; j in 2,3 ->
                        # i-chunk c=2g+(j-2).  wo rows: 0:1024 r (chunk c at 128c),
                        # 1024:2048 i, 2048:3072 sum.
                        with (
                            tc.tile_pool(name="ocp", bufs=1) as ocp,
                            tc.tile_pool(name="yps", bufs=2, space="PSUM") as yps,
                            tc.tile_pool(name="ysb", bufs=4) as ysbp,
                        ):
                            oc_r, oc_i, oc_s = [], [], []
                            for c in range(NK):
                                g, j = divmod(c, 2)
                                tr = ocp.tile([128, LL], F16, tag=f"ocr{c}", name=f"ocr{c}")
                                nc.sync.dma_start(
                                    tr[:], ag_out[(4 * g + j) * 128:(4 * g + j + 1) * 128, :])
                                oc_r.append(tr)
                                ti = ocp.tile([128, LL], F16, tag=f"oci{c}", name=f"oci{c}")
                                nc.sync.dma_start(
                                    ti[:], ag_out[(4 * g + 2 + j) * 128:(4 * g + 3 + j) * 128, :])
                                oc_i.append(ti)
                                ts = ocp.tile([128, LL], F16, tag=f"ocs{c}", name=f"ocs{c}")
                                eng = nc.vector if c % 2 == 0 else nc.gpsimd
                                eng.tensor_tensor(ts[:], tr[:], ti[:], ADD)
                                oc_s.append(ts)

                            for n in range(NLC):
                                col = slice(n * 512, (n + 1) * 512)
                                for g in range(2):
                                    cs = slice(128 * g, 128 * g + 128)
                                    m1 = yps.tile([128, 512], F32, tag="m1", name="m1")
                                    for k in range(NK):
                                        nc.tensor.matmul(
                                            m1[:], wo_t[k][:, cs], oc_r[k][:, col],
                                            start=(k == 0), stop=(k == NK - 1))
                                    m2 = yps.tile([128, 512], F32, tag="m2", name="m2")
                                    for k in range(NK):
                                        nc.tensor.matmul(
                                            m2[:], wo_t[NK + k][:, cs],
                                            oc_i[k][:, col],
                                            start=(k == 0), stop=(k == NK - 1))
                                    m3 = yps.tile([128, 512], F32, tag="m3", name="m3")
                                    for k in range(NK):
                                        nc.tensor.matmul(
                                            m3[:], wo_t[2 * NK + k][:, cs], oc_s[k][:, col],
                                            start=(k == 0), stop=(k == NK - 1))
                                    s1 = ysbp.tile([128, 512], F32, tag="s1", name="s1")
                                    nc.scalar.activation(
                                        s1[:], m1[:],
                                        mybir.ActivationFunctionType.Identity)
                                    yr = ysbp.tile([128, 512], F16, tag="yr", name="yr")
                                    nc.vector.scalar_tensor_tensor(
                                        yr[:], s1[:], bo_t[:, g:g + 1], m2[:],
                                        op0=ADD, op1=SUB)
                                    nc.sync.dma_start(yT[g * 128:(g + 1) * 128, col], yr[:])
                                    tm = ysbp.tile([128, 512], F32, tag="ytm", name="ytm")
                                    nc.vector.tensor_tensor(tm[:], s1[:], m2[:], ADD)
                                    yi = ysbp.tile([128, 512], F16, tag="yi", name="yi")
                                    nc.vector.scalar_tensor_tensor(
                                        yi[:], m3[:], bo_t[:, 2 + g:3 + g], tm[:],
                                        op0=ADD, op1=SUB)
                                    nc.sync.dma_start(
                                        yT[256 + g * 128:256 + (g + 1) * 128, col], yi[:])

    if compile:
        nc.compile()
    return nc


def _get(seq_len=L, repeat=1, with_cc=True):
    key = (seq_len, repeat, with_cc)
    if key not in _CACHE:
        _CACHE[key] = _build(seq_len, repeat=repeat, with_cc=with_cc)
    return _CACHE[key]


def _prep_inputs(x_real, x_imag, wq_r, wq_i, wk_r, wk_i, wv_r, wv_i,
                 wo_r, wo_i, bo_r, bo_i):
    """Host-side sharding: per-core input maps (fp16 layout prep)."""
    f16 = np.float16
    seq_len = x_real.shape[1]

    xr_b, xi_b = [], []
    for b in range(B):
        xr_b.append(np.ascontiguousarray(x_real[b].T).astype(f16))
        xi_b.append(np.ascontiguousarray(x_imag[b].T).astype(f16))

    mask01 = np.triu(np.ones((128, 128), dtype=np.float32)).astype(f16)
    ident = np.eye(128, dtype=np.float32).astype(f16)
    ones = np.ones((128, 1), dtype=f16)

    def proj_eff(w_r, w_i, heads):
        """Karatsuba weights [3D, 256]: rows (Wr ; Wi ; Wr+Wi), cols by
        head pair [h0 | h1] then [h2 | h3]."""
        w_eff = np.empty((D3, 256), dtype=np.float32)
        w_s = w_r + w_i
        for t, h in enumerate(heads):
            c0 = 64 * t
            rows = slice(64 * h, 64 * h + 64)
            w_eff[:D, c0:c0 + 64] = w_r[rows, :].T
            w_eff[D:2 * D, c0:c0 + 64] = w_i[rows, :].T
            w_eff[2 * D:, c0:c0 + 64] = w_s[rows, :].T
        return w_eff.astype(f16)

    in_maps = []
    for c in range(NCORES):
        b, g = divmod(c, GROUP)
        heads = [4 * g + t for t in range(NH)]
        ycols = slice(256 * g, 256 * g + 256)

        wq_eff = proj_eff(wq_r, wq_i, heads)
        wk_eff = proj_eff(wk_r, wk_i, heads)
        wv_eff = proj_eff(wv_r, wv_i, heads)

        # out-proj Karatsuba weights [3D, 256]: contraction rows follow the
        # AllGather layout remapped to (all-r ; all-i ; all-sum) chunks.
        wo_eff = np.empty((D3, 256), dtype=np.float32)
        wo_s = wo_r + wo_i
        wo_eff[:D, :] = wo_r[ycols, :].T
        wo_eff[D:2 * D, :] = wo_i[ycols, :].T
        wo_eff[2 * D:, :] = wo_s[ycols, :].T
        wo_eff = wo_eff.astype(f16)

        bo_eff = np.concatenate(
            [bo_r[ycols], bo_i[ycols]]).astype(np.float32).reshape(JC, 1)

        in_maps.append({
            "xr": xr_b[b], "xi": xi_b[b], "wq": wq_eff, "wk": wk_eff,
            "wv": wv_eff, "wo": wo_eff, "bo": bo_eff, "mask": mask01,
            "ident": ident, "ones": ones,
        })
    return in_maps, seq_len


def _run(in_maps, seq_len):
    nc = _get(seq_len)
    res = run_bass_kernel_spmd(nc, in_maps, core_ids=list(range(NCORES)),
                               trace=False)
    return res


def _assemble(results, seq_len):
    yr = np.empty((B, seq_len, D), dtype=np.float32)
    yi = np.empty((B, seq_len, D), dtype=np.float32)
    for c in range(NCORES):
        b, g = divmod(c, GROUP)
        yT_c = results[c]["yT"]                      # [512, LL] fp16
        yr[b][:, 256 * g:256 * g + 256] = yT_c[:256].T.astype(np.float32)
        yi[b][:, 256 * g:256 * g + 256] = yT_c[256:].T.astype(np.float32)
    return yr, yi


def kernel(x_real, x_imag, wq_r, wq_i, wk_r, wk_i, wv_r, wv_i,
           wo_r, wo_i, bo_r, bo_i):
    args = [np.asarray(a) for a in (x_real, x_imag, wq_r, wq_i, wk_r, wk_i,
                                    wv_r, wv_i, wo_r, wo_i, bo_r, bo_i)]
    in_maps, seq_len = _prep_inputs(*args)
    res = _run(in_maps, seq_len)
    return _assemble(res.results, seq_len)
